# revision 51
# baseline (speedup 1.0000x reference)
"""Head-sharded causal GQA prefill attention on 8 TRN2 NeuronCores.

Problem: B=2, S=2048, H=32 query heads, HKV=8 kv heads, D=128.
Sharding: kv head h -> core h (4 query heads + 1 kv head per core);
no cross-core communication inside attention.

Per-core algorithm (per (q-head, batch) "head-batch", 8 of them):
  - scores are computed TRANSPOSED: S^T[k, q] = K @ Q^T via TensorE with
    kT block as stationary weights and qT chunk (512 q) as moving operand.
  - exp is split across ScalarE and VectorE: ScalarE does the diagonal
    strips (which carry the largest attention weights) and 6/12 full-strip
    pairs as exact exp straight out of PSUM; VectorE does the other 6
    pairs with a ONE-PASS Schraudolph: a single tensor_scalar computes
    int16(A16*s + B16), which IS the bf16 bit pattern of exp(SCALE*s)
    (f32 Schraudolph bits / 2^16), written through a bf16-tile bitcast —
    no second bitcast/copy pass. This balances the engines at ~100us each
    per core, both under the PE's ~135us.
  - PV uses the P^T block as stationary weights against rhs [V | ones]
    (129 cols) so the softmax row-sum accumulates for free in column 128.
  - two q-blocks share one PSUM bank ([128, 2, 132] tiles, 8B-aligned
    slots), so normalization is batched: one reciprocal + one broadcast
    tensor_mul per PAIR of q-blocks; output DMAs go out per q-block pair
    so the kernel tail only waits on the final pair's normalize.

Causality is exact at 128-block granularity: blocks with k_block > q_block
are skipped and the QK matmuls of the 4 diagonal strips of each chunk are
narrowed to the valid q range. The diagonal 128x128 blocks of each chunk
get an upper-triangular bf16 mask post-exp: ONE strided [128,2,128]
tensor_mul per diag pair tile, emitted lazily just before the PV chains
that consume it so the VectorE queue never blocks behind ScalarE.

PE scheduling: stage s+1's QK matmuls are INTERLEAVED with stage s's PV
chains (diag, pv0, pv1, 2 pairs, pv2, 2 pairs, pv3, rest) so the PE always
has PV work while the exp engines drain score PSUM tiles, instead of
stalling on the 3-deep score-PSUM pool during a QK burst. Input DMAs are
prefetched 1-2 stages ahead (the second batch's K/V a full head early) and
a 40-matmul dummy warmup keeps the PE busy through the initial DMA window
so the HAM clock gate reaches 8/8 before real work starts.
"""

import sys

sys.path.insert(0, "/opt/trn_rl_repo")

import numpy as np
from ml_dtypes import bfloat16

B, S = 2, 2048
H, HKV, D = 32, 8, 128
G = H // HKV  # 4 query heads per kv head
NCORES = 8
SCALE = 0.08838834764831845
NQB = S // 128  # 16 q/k blocks per sequence
NCH = 4  # q chunks of 512

# The 4 narrowed diagonal strips (widths 512, 384, 256, 128) pack into two
# [128,1024] pair tiles: tile A holds m0 [0:512) + m1 [512:896); tile B holds
# m2 [0:256) + m3 [256:384). Each strip stays within one PSUM bank.
DIAG_W = [512, 384, 256, 128]

# One-pass Schraudolph: bf16 bits of exp(SCALE*s) ~= int16(A16*s + B16)
# (f32 Schraudolph bits / 2^16 == the bf16 bit pattern; one tensor_scalar,
# no second bitcast/copy pass). B16 tuned numerically against the real
# score distribution (min final-output rel err, truncating f32->i16).
SCH_A16 = SCALE * 1.4426950408889634 * 128.0
SCH_B16 = 16255.5

# full-strip pair indices computed on VectorE (Schraudolph 1-pass), per
# chunk c; the rest (and all diagonal strips, which carry the largest
# attention weights) use exact exp on ScalarE. 6/12 pairs on VectorE
# balances the two engines at ~100us per core each.
V_PAIRS = {3: (1, 3), 2: (1, 3), 1: (0, 1), 0: ()}

N_WARMUP = 33  # dummy 128-col matmuls covering the initial DMA window

_CACHE = {}
_RUN_KWARGS = {}  # test harness may set e.g. {"trace": True, "tmpdir": ...}


def _build_nc():
    import concourse.mybir as mybir
    import concourse.tile as tile
    from concourse import bacc
    from concourse.masks import make_upper_triangular

    f32 = mybir.dt.float32
    bf16 = mybir.dt.bfloat16
    i16 = mybir.dt.int16
    EXP = mybir.ActivationFunctionType.Exp

    nc = bacc.Bacc("TRN2", target_bir_lowering=False, debug=False, num_devices=NCORES)

    qT = nc.declare_dram_parameter("qt", [G * B, 128, S], bf16, isOutput=False)
    kT = nc.declare_dram_parameter("kt", [B, 128, S], bf16, isOutput=False)
    vo = nc.declare_dram_parameter("vo", [B, 128, NQB, 129], bf16, isOutput=False)
    o = nc.declare_dram_parameter("o", [G * B, 128, NQB, 128], f32, isOutput=True)

    from contextlib import ExitStack

    with tile.TileContext(nc) as tc, ExitStack() as ctx:
        consts = ctx.enter_context(tc.tile_pool(name="consts", bufs=1))
        kpool = ctx.enter_context(tc.tile_pool(name="kpool", bufs=2))
        vpool = ctx.enter_context(tc.tile_pool(name="vpool", bufs=2))
        qpool = ctx.enter_context(tc.tile_pool(name="qpool", bufs=2))
        opool = ctx.enter_context(tc.tile_pool(name="opool", bufs=4))
        ptpool = ctx.enter_context(tc.tile_pool(name="ptpool", bufs=16))
        rpool = ctx.enter_context(tc.tile_pool(name="rpool", bufs=8))
        spsum = ctx.enter_context(tc.tile_pool(name="spsum", bufs=3, space="PSUM"))
        opsum = ctx.enter_context(tc.tile_pool(name="opsum", bufs=2, space="PSUM"))

        # HAM warmup: matmuls gated only on a cheap memset run during the
        # input-DMA window so the PE clock gate reaches 8/8 before real work.
        dummy = consts.tile([128, 128], bf16)
        nc.vector.memset(dummy, 0.0)
        # ... and a 1-element dummy exp so walrus's ACT_TABLE_LOAD (~1.3us +
        # drain) runs on ScalarE during the DMA window instead of inside the
        # first real score-exp's critical path.
        tl = consts.tile([128, 1], f32, name="tl")
        nc.vector.memset(tl, 0.0)
        nc.scalar.activation(out=tl, in_=tl, func=EXP, scale=1.0)
        warm = opsum.tile([128, 2, 132], f32, name="warm", tag="ops")
        for _ in range(N_WARMUP):
            nc.tensor.matmul(
                warm[:, 0, 0:128], lhsT=dummy, rhs=dummy, start=True, stop=True
            )

        # Upper-triangular (k <= q) 0/1 mask, duplicated so one strided
        # tensor_mul can mask both diagonal blocks of a pair tile at once.
        mask_f = consts.tile([128, 128], f32)
        make_upper_triangular(nc, mask_f, val=1.0, diag=True)
        mask2 = consts.tile([128, 2, 128], bf16)
        nc.vector.tensor_copy(mask2[:, 0, :], mask_f)
        nc.vector.tensor_copy(mask2[:, 1, :], mask_f)

        # stage list: chunk-descending inside each (batch, head) so the final
        # stage has the smallest PV tail
        stages = []
        for b in range(B):
            for g in range(G):
                for c in range(NCH - 1, -1, -1):
                    stages.append((b, g, c))

        kt_sb = [None] * B
        vo_sb = [None] * B
        state = {}  # (b, g) -> {"qt": tile}
        # strip record: (stage_idx, k_block_j) -> (pt_tile, base_col)
        # lhsT for q sub-block m is pt_tile[:, base + 128*m : base + 128*m+128]
        strips = {}
        diag_info = {}  # stage -> diag tiles, for lazy batched masks

        def emit_masks(s):
            # mask the diagonal 128x128 block of each diagonal strip with ONE
            # strided tensor_mul per pair tile ([128, 2, 128] view covering
            # both diag blocks); emitted lazily (just before stage s's PV) so
            # VectorE never sits blocked behind ScalarE's diag exp.
            from concourse.bass import AP

            for (psd, ptd, width), stride in zip(diag_info.pop(s), (512, 256)):
                base = ptd[:, 0:128]
                v = AP(base.tensor, base.offset, [base.ap[0], [stride, 2], [1, 128]])
                nc.vector.tensor_mul(v, v, mask2)

        def ensure_inputs(s, first=False):
            """Allocate + DMA the SBUF input tiles stage s needs (idempotent)."""
            b, g, c = stages[s]
            if kt_sb[b] is None:
                kt_sb[b] = kpool.tile([128, S], bf16, name="kt_sb")
                if first:
                    # tail columns first: stage (b0,g0,c3)'s diag reads them
                    nc.sync.dma_start(
                        out=kt_sb[b][:, 1536:2048], in_=kT[b, :, 1536:2048]
                    )
                else:
                    nc.sync.dma_start(out=kt_sb[b], in_=kT[b, :, :])
            if (b, g) not in state:
                qt = qpool.tile([128, S], bf16, name="qt_sb")
                if first:
                    nc.sync.dma_start(
                        out=qt[:, 1536:2048], in_=qT[g * B + b, :, 1536:2048]
                    )
                    # rest of the critical pair, then bulk
                    nc.sync.dma_start(out=kt_sb[b][:, 0:1536], in_=kT[b, :, 0:1536])
                else:
                    nc.sync.dma_start(out=qt, in_=qT[g * B + b, :, :])
                state[(b, g)] = {"qt": qt}
            if vo_sb[b] is None:
                vo_sb[b] = vpool.tile([128, NQB, 129], bf16, name="vo_sb")
                nc.sync.dma_start(out=vo_sb[b], in_=vo[b, :, :, :])
            if first:
                nc.sync.dma_start(
                    out=state[(b, g)]["qt"][:, 0:1536], in_=qT[g * B + b, :, 0:1536]
                )

        def exp_tile(ps, pt, width, on_vector):
            """exp a [128, width] PSUM score tile into the bf16 pt tile."""
            if on_vector:
                # one-pass Schraudolph: bf16 exp bits as int16(A16*s + B16)
                nc.vector.tensor_scalar(
                    out=pt[:, 0:width].bitcast(i16),
                    in0=ps[:, 0:width],
                    scalar1=float(SCH_A16),
                    scalar2=float(SCH_B16),
                    op0=mybir.AluOpType.mult,
                    op1=mybir.AluOpType.add,
                )
            else:
                nc.scalar.activation(
                    out=pt[:, 0:width], in_=ps[:, 0:width], func=EXP, scale=SCALE
                )

        def emit_diag(s):
            b, g, c = stages[s]
            ensure_inputs(s, first=(s == 0))
            # prefetch: next head's qt two stages early; next batch's K/V a
            # full head early; next stage's inputs as fallback
            if c == 1 and s + 2 < len(stages):
                ensure_inputs(s + 2)
            if s + 1 < len(stages):
                ensure_inputs(s + 1)
            if (b, g, c) == (0, G - 1, NCH - 1) and B > 1:
                kt_sb[1] = kpool.tile([128, S], bf16, name="kt_sb")
                nc.sync.dma_start(out=kt_sb[1], in_=kT[1, :, :])
                vo_sb[1] = vpool.tile([128, NQB, 129], bf16, name="vo_sb")
                nc.sync.dma_start(out=vo_sb[1], in_=vo[1, :, :, :])

            qt = state[(b, g)]["qt"]
            # (psum_tile_idx, psum_col) per diagonal strip; two pair tiles
            packs = [(0, 0), (0, 512), (1, 0), (1, 256)]
            tiles = []
            for t, width in ((0, 896), (1, 384)):
                psd = spsum.tile([128, 1024], f32, name="psd", tag="ps")
                ptd = ptpool.tile([128, 1024], bf16, name="ptd", tag="pt")
                tiles.append((psd, ptd, width))
            for m in range(4):
                j = 4 * c + m
                t, col = packs[m]
                psd, ptd, _ = tiles[t]
                nc.tensor.matmul(
                    psd[:, col : col + DIAG_W[m]],
                    lhsT=kt_sb[b][:, j * 128 : (j + 1) * 128],
                    rhs=qt[:, c * 512 + 128 * m : (c + 1) * 512],
                    start=True,
                    stop=True,
                )
                strips[(s, j)] = (ptd, col - 128 * m)
            for psd, ptd, width in tiles:
                exp_tile(psd, ptd, width, False)  # diags: exact exp (ScalarE)
            diag_info[s] = tiles

        def emit_pair(s, p):
            """QK + exp for full strips j = 2p, 2p+1 of stage s."""
            b, g, c = stages[s]
            ps = spsum.tile([128, 1024], f32, name="ps2", tag="ps")
            pt = ptpool.tile([128, 1024], bf16, name="pt2", tag="pt")
            for slot in range(2):
                j = 2 * p + slot
                nc.tensor.matmul(
                    ps[:, slot * 512 : (slot + 1) * 512],
                    lhsT=kt_sb[b][:, j * 128 : (j + 1) * 128],
                    rhs=state[(b, g)]["qt"][:, c * 512 : (c + 1) * 512],
                    start=True,
                    stop=True,
                )
                strips[(s, j)] = (pt, slot * 512)
            exp_tile(ps, pt, 1024, p in V_PAIRS[c])

        def make_pv_parts(s):
            b, g, c = stages[s]
            ctx = {}

            def chain(m):
                qb = 4 * c + m
                if m % 2 == 0:
                    ctx[m // 2] = opsum.tile([128, 2, 132], f32, name="ops", tag="ops")
                ops = ctx[m // 2]
                for j in range(qb + 1):
                    pt, base = strips[(s, j)]
                    nc.tensor.matmul(
                        ops[:, m % 2, 0:129],
                        lhsT=pt[:, base + 128 * m : base + 128 * m + 128],
                        rhs=vo_sb[b][:, j, :],
                        start=(j == 0),
                        stop=(j == qb),
                    )
                if m % 2 == 1:
                    t = m // 2
                    if t == 0:
                        ctx["osb"] = opool.tile([128, 4, 128], f32, name="o_sb")
                    rec = rpool.tile([128, 2, 1], f32, name="rec")
                    nc.vector.reciprocal_approx_fast(rec, ops[:, :, 128:129])
                    nc.vector.tensor_mul(
                        ctx["osb"][:, 2 * t : 2 * t + 2, :],
                        ops[:, :, 0:128],
                        rec[:, :, 0:1].broadcast_to([128, 2, 128]),
                    )
                    # DMA out per q-block pair so the kernel tail only waits
                    # on the final pair's normalize, not the whole chunk's
                    nc.sync.dma_start(
                        out=o[g * B + b, :, 4 * c + 2 * t : 4 * c + 2 * t + 2, :],
                        in_=ctx["osb"][:, 2 * t : 2 * t + 2, :],
                    )
                if m == 3:
                    for j in range(4 * c + 4):
                        del strips[(s, j)]

            return [lambda m=m: chain(m) for m in range(4)]

        # ---- main emission: stage s+1's QK interleaved with stage s's PV ----
        def qk_stage(s):
            emit_diag(s)
            return [lambda p=p: emit_pair(s, p) for p in range(2 * stages[s][2])]

        for p_fn in qk_stage(0):
            p_fn()
        for s in range(len(stages)):
            emit_masks(s)
            pv = make_pv_parts(s)
            # same-head next stage: its qt is already resident, so emit its
            # diag QK before pv0 — ScalarE gets the diag exp ~0.7us earlier.
            # New-head next stage: pv0 first so a late qt DMA hides behind PV.
            same_head = s + 1 < len(stages) and stages[s + 1][:2] == stages[s][:2]
            if same_head:
                pairs = qk_stage(s + 1)
                pv[0]()
            else:
                pv[0]()
                pairs = qk_stage(s + 1) if s + 1 < len(stages) else []
            pv[1]()
            for p_fn in pairs[0:2]:
                p_fn()
            pv[2]()
            for p_fn in pairs[2:4]:
                p_fn()
            pv[3]()
            for p_fn in pairs[4:]:
                p_fn()

    nc.compile()
    return nc


def _get_nc():
    if "nc" not in _CACHE:
        _CACHE["nc"] = _build_nc()
    return _CACHE["nc"]


def kernel(q, k, v):
    from concourse.bass_utils import run_bass_kernel_spmd

    assert q.shape == (B * S, H * D) and k.shape == (B * S, HKV * D)
    nc = _get_nc()

    in_maps = []
    for c in range(NCORES):
        qc = q[:, c * G * D : (c + 1) * G * D].reshape(B, S, G, D)
        qt = np.ascontiguousarray(qc.transpose(2, 0, 3, 1)).reshape(G * B, D, S)
        kc = k[:, c * D : (c + 1) * D].reshape(B, S, D)
        kt = np.ascontiguousarray(kc.transpose(0, 2, 1))
        vc = v[:, c * D : (c + 1) * D].reshape(B, NQB, 128, D)
        vones = np.ones((B, 128, NQB, D + 1), dtype=np.float32)
        vones[:, :, :, :D] = vc.transpose(0, 2, 1, 3)
        in_maps.append(
            {
                "qt": qt.astype(bfloat16),
                "kt": kt.astype(bfloat16),
                "vo": vones.astype(bfloat16),
            }
        )

    res = run_bass_kernel_spmd(
        nc, in_maps, core_ids=list(range(NCORES)), **_RUN_KWARGS
    )
    _CACHE["last_result"] = res

    out = np.empty((B * S, H * D), dtype=np.float32)
    for c in range(NCORES):
        oc = res.results[c]["o"].reshape(G, B, 128, NQB, 128)
        # o[g, b, p, n, d] -> out[b*S + n*128 + p, c*512 + g*128 + d]
        out[:, c * G * D : (c + 1) * G * D] = (
            oc.transpose(1, 3, 2, 0, 4).reshape(B * S, G * D)
        )
    return out


if __name__ == "__main__":
    rng = np.random.default_rng(0)
    q = rng.standard_normal((B * S, H * D), dtype=np.float32)
    k = rng.standard_normal((B * S, HKV * D), dtype=np.float32)
    v = rng.standard_normal((B * S, HKV * D), dtype=np.float32)
    out = kernel(q, k, v)
    print(out.shape, out.dtype)
